# revision 31
# baseline (speedup 1.0000x reference)
"""Trainium2 Bass kernel for nn_PartialConvLayer (partial conv 3x3 + mask
update + BatchNorm(batch stats) + ReLU), data-parallel over batch on 8 cores.

Math (per image):
  update = conv(mask, ones(Cin,3,3)), pad 1          # integer in {0..576}
  u      = clip(update, 0, 1)                        # exactly binary
  mr     = 576 / (update + 1e-6) * u
  conv   = conv(x*mask, W), pad 1                    # no bias
  prebn  = conv * mr * u + b * u
  BN over (N,H,W) batch stats (all-reduced across cores), then ReLU.
Returns (out, broadcast(update_clipped)).

v3 design (on top of v2's balanced DMA + SBUF-resident bf16 prebn):
  - 128-partition contraction matmuls run ~1.7x faster than 64 on TRN2's PE,
    so the conv uses a dual xm layout: partitions 0:64 hold x*m with a left
    guard column (tap kx reads window [kx, kx+256)), partitions 64:128 hold
    the same band's x*m unshifted (same window gives tap kx+1). Taps pair as
    (ky,0)+(ky,1) in one 128-contraction matmul; (ky,2) stays 64-contraction.
    The duplicate halves come from loading x and mask twice per half-block.
  - Each half-block = one band x 8 rows = 4 chunks of [COUT, 512].
  - Per-chunk mask scalars via zero-padded 128-row selector matmuls:
    e8[p, q*128+o] = d(p,q) broadcasts mru pair-strips (mru8[128,512],
    rows 0:8 = strips, rest zeros) into PSUM in ONE matmul; bps8 folds
    b[o]/576 * v into the conv PSUM in ONE matmul.
  - Update path (ones-matmul channel sums -> sb->sb scatter -> T3 vertical
    sum -> DVE math) runs one block ahead of the conv consuming it.
"""
import os
import numpy as np
from contextlib import ExitStack

import concourse.bass as bass
import concourse.tile as tile
from concourse import mybir, bacc
from concourse import library_config
from concourse.bass_utils import run_bass_kernel_spmd

F32 = mybir.dt.float32
F32R = mybir.dt.float32r
BF16 = mybir.dt.bfloat16
ALU = mybir.AluOpType
ACTF = mybir.ActivationFunctionType

CIN = 64
COUT = 128
W_ = 256
KS = 3
EPS_MASK = 1e-6
EPS_BN = 1e-5
SLIDE = float(CIN * KS * KS)   # 576


def build_nc(n_cores=8, H=256, B=8):
    """SPMD program for one core holding one [CIN, H, W_] image."""
    HB = H // 2                      # rows per band
    nblk = HB // B                   # 16 blocks
    nhb = 2 * nblk                   # 32 half-blocks (block K, band b)
    nrows = B + 2                    # rows per band tile (with halo)
    nchunk = (H * W_) // 512         # 128 chunks per core
    TOT = float(n_cores * H * W_)    # BN count

    nc = bacc.Bacc(None, num_devices=n_cores)

    X = nc.dram_tensor("x", [CIN, H, W_], F32, kind="ExternalInput")
    M = nc.dram_tensor("mask", [CIN, H, W_], F32R, kind="ExternalInput")
    # wtp [128, 384] (tap pairs) | wts [64, 384] (kx=2 singles) packed
    WTS = nc.dram_tensor("wts", [128, 768], F32R, kind="ExternalInput")
    EB1 = nc.dram_tensor("eb1", [128, 2048], F32R, kind="ExternalInput")
    EB2 = nc.dram_tensor("eb2", [128, 2048], F32R, kind="ExternalInput")
    HALF = nc.dram_tensor("half", [128, 2], F32R, kind="ExternalInput")
    T3 = nc.dram_tensor("t3", [2 * nrows, 2 * B], F32R, kind="ExternalInput")
    GAM = nc.dram_tensor("gam", [COUT, 1], F32, kind="ExternalInput")
    BET = nc.dram_tensor("bet", [COUT, 1], F32, kind="ExternalInput")

    OUT = nc.dram_tensor("out", [COUT, H * W_], F32, kind="ExternalOutput")
    DBG = (nc.dram_tensor("dbg", [COUT, H * W_], F32, kind="ExternalOutput")
           if os.environ.get("DEBUG_PREBN") else None)

    UPD = nc.dram_tensor("upd", [H, W_], F32, kind="ExternalOutput")

    cc_in = nc.dram_tensor("ccin", [COUT, 2], F32)
    cc_out = nc.dram_tensor("ccout", [COUT, 2], F32,
                            addr_space="Shared" if n_cores > 4 else "Local")

    with tile.TileContext(nc) as tc, ExitStack() as ctx:
        nc.gpsimd.load_library(library_config.mlp)

        const = ctx.enter_context(tc.tile_pool(name="const", bufs=1))
        iox = ctx.enter_context(tc.tile_pool(name="iox", bufs=1))
        iom = ctx.enter_context(tc.tile_pool(name="iom", bufs=3))
        up1 = ctx.enter_context(tc.tile_pool(name="up1", bufs=1))
        up2 = ctx.enter_context(tc.tile_pool(name="up2", bufs=2))
        chk = ctx.enter_context(tc.tile_pool(name="chk", bufs=1))
        mb = ctx.enter_context(tc.tile_pool(name="mb", bufs=2))
        psc = ctx.enter_context(tc.tile_pool(name="psc", bufs=3, space="PSUM"))
        psb = ctx.enter_context(tc.tile_pool(name="psb", bufs=3, space="PSUM"))
        pss = ctx.enter_context(tc.tile_pool(name="pss", bufs=1, space="PSUM"))
        psu = ctx.enter_context(tc.tile_pool(name="psu", bufs=1, space="PSUM"))

        # ---- first x load starts immediately; consts stage via m slots ----
        # x/m staged pre-padded and pre-shifted: [0:64] data at col 1 (guards
        # 0,257), [64:128] data at col 0 (guards 256,257), 258-pitch rows.
        def load_dup(tens, tl, k, b, first):
            r0 = k * B
            base = r0 + b * HB
            lo = max(base - 1, 0)
            hi = min(base + B + 1, H)
            n = hi - lo
            sl0 = lo - (base - 1)
            if first:
                nc.vector.memset(tl[:, :].bitcast(F32), 0.0)
            tl3 = tl[:, :].rearrange("p (r c) -> p r c", c=258)
            for h in range(2):
                nc.sync.dma_start(
                    tl3[64 * h:64 * h + 64, sl0:sl0 + n, 1 - h:257 - h],
                    bass.AP(tensor=tens, offset=lo * W_,
                            ap=[[H * W_, CIN], [W_, n], [1, W_]]))
            if not first:
                if sl0 > 0:
                    nc.vector.memset(tl[:, 0:sl0 * 258].bitcast(F32), 0.0)
                if sl0 + n < nrows:
                    nc.vector.memset(
                        tl[:, (sl0 + n) * 258:nrows * 258].bitcast(F32), 0.0)

        def load_x(hb):
            x_t = iox.tile([128, nrows * 258], F32, tag="x_t")
            load_dup(X, x_t, hb // 2, hb % 2, hb < 1)
            return x_t

        def load_m(hb):
            m_t = iom.tile([128, nrows * 258], F32R, tag="m_t")
            load_dup(M, m_t, hb // 2, hb % 2, hb < 3)
            return m_t

        m_first = [None]

        # m(0) + x(0) loads start immediately; consts stage via later slots
        m_first[0] = load_m(0)
        x_tiles = {0: load_x(0)}
        stg = iom.tile([128, nrows * 258], F32R, tag="m_t")
        nc.sync.dma_start(stg[:, 0:768], WTS[:])
        wtp_b = const.tile([128, KS * COUT], BF16)
        nc.vector.tensor_copy(wtp_b[:], stg[:, 0:KS * COUT])
        wts_b = const.tile([64, KS * COUT], BF16)
        nc.vector.tensor_copy(wts_b[:], stg[0:64, KS * COUT:2 * KS * COUT])
        stg2 = iom.tile([128, nrows * 258], F32R, tag="m_t")
        nc.sync.dma_start(stg2[:, 0:2048], EB1[:])
        e16_b = const.tile([128, 16 * COUT], BF16)
        nc.vector.tensor_copy(e16_b[:], stg2[:, 0:16 * COUT])
        stg3 = iom.tile([128, nrows * 258], F32R, tag="m_t")
        nc.sync.dma_start(stg3[:, 0:2048], EB2[:])
        bps16_b = const.tile([128, 16 * COUT], BF16)
        nc.vector.tensor_copy(bps16_b[:], stg3[:, 0:16 * COUT])
        t3_r = const.tile([2 * nrows, 2 * B], F32R)
        nc.sync.dma_start(t3_r[:], T3[:])

        half_t = const.tile([128, 2], F32R)
        nc.sync.dma_start(half_t[:], HALF[:])
        gam_t = const.tile([COUT, 1], F32)
        nc.sync.dma_start(gam_t[:], GAM[:])
        bet_t = const.tile([COUT, 1], F32)
        nc.sync.dma_start(bet_t[:], BET[:])
        eps_t = const.tile([COUT, 1], F32)
        nc.vector.memset(eps_t[:], EPS_BN)
        sum_slots = const.tile([COUT, nchunk], F32)
        sq_slots = const.tile([COUT, nchunk], F32)

        # dual xm ring: top half guard cols {0,257}, bottom {256,257}
        xm_tiles = []
        for i in range(2):
            t = const.tile([128, nrows * 258], BF16, tag=f"xm{i}")
            nc.vector.memset(t[:].bitcast(F32), 0.0)
            xm_tiles.append(t)
        s_rows = const.tile([2 * nrows, 258], F32R)
        nc.vector.memset(s_rows[:].bitcast(F32), 0.0)

        prebn = const.tile([128, H * W_], BF16)

        m_tiles = {0: m_first[0], 1: load_m(1), 2: load_m(2)}

        def upd_part1(hb):
            """Channel sums of mask band hb -> scatter into s_rows rows."""
            b = hb % 2
            m3 = m_tiles[hb][:, :].rearrange("p (r c) -> p r c", c=258)
            for p in range(nrows // 2):
                ps_s = pss.tile([2, 512], F32, tag="ps_s")
                nc.tensor.matmul(ps_s[:], half_t[:],
                                 m3[:, 2 * p:2 * p + 2, 1:257],
                                 start=True, stop=True)
                s_pair = up1.tile([1, 512], F32R, tag="s_pair")
                nc.vector.tensor_copy(s_pair[:], ps_s[0:1, :])
                nc.sync.dma_start(
                    s_rows[b * nrows + 2 * p:b * nrows + 2 * p + 2, 1:257],
                    s_pair[0:1, :].rearrange("p (r c) -> p r c", c=256))

        def upd_t3():
            ps_u = psu.tile([2 * B, 258], F32, tag="ps_u")
            nc.tensor.matmul(ps_u[:], t3_r[:], s_rows[:], start=True, stop=True)
            return ps_u

        def upd_part2(K, ps_u):
            """Horizontal sum + update math on [16, 256] for block K."""
            r0 = K * B
            u_sb = up1.tile([2 * B, 258], F32, tag="u_sb")
            nc.scalar.copy(u_sb[:], ps_u[:])
            vh = up1.tile([2 * B, W_], F32, tag="vh")
            nc.vector.tensor_add(vh[:], u_sb[:, 0:256], u_sb[:, 1:257])
            nc.vector.tensor_add(vh[:], vh[:], u_sb[:, 2:258])
            u_clip = up1.tile([2 * B, W_], F32, tag="u_clip")
            nc.vector.tensor_scalar_min(u_clip[:], vh[:], 1.0)
            upde = up1.tile([2 * B, W_], F32, tag="upde")
            nc.vector.tensor_scalar_add(upde[:], vh[:], EPS_MASK)
            rec = up1.tile([2 * B, W_], F32, tag="vh")
            nc.vector.reciprocal(rec[:], upde[:])
            mru16f = up1.tile([2 * B, W_], F32, tag="mru16f")
            nc.vector.scalar_tensor_tensor(
                out=mru16f[:], in0=rec[:], scalar=SLIDE, in1=u_clip[:],
                op0=ALU.mult, op1=ALU.mult)
            v16f = up1.tile([2 * B, W_], F32, tag="v16f")
            nc.vector.scalar_tensor_tensor(
                out=v16f[:], in0=upde[:], scalar=1.0, in1=u_clip[:],
                op0=ALU.mult, op1=ALU.mult)
            mru16 = up2.tile([128, W_], BF16, tag="mru16")
            if K < 2:
                nc.vector.memset(mru16[:].bitcast(F32), 0.0)
            nc.vector.tensor_copy(mru16[0:2 * B, :], mru16f[:])
            v16 = up2.tile([128, W_], BF16, tag="v16")
            if K < 2:
                nc.vector.memset(v16[:].bitcast(F32), 0.0)
            nc.vector.tensor_copy(v16[0:2 * B, :], v16f[:])
            nc.sync.dma_start(
                bass.AP(tensor=UPD, offset=r0 * W_,
                        ap=[[HB * W_, 2], [W_, B], [1, W_]]),
                u_clip[:])
            return mru16, v16

        def make_xm(hb, half):
            """One contiguous full-width multiply (inputs pre-shifted)."""
            if half != 0:
                return xm_tiles[hb % 2]
            xm = xm_tiles[hb % 2]
            nc.vector.tensor_tensor(
                xm[:, :], x_tiles[hb][:, :], m_tiles[hb][:, :], op=ALU.mult)
            return xm

        def bcast_chunk(hb, j, mru16):
            b = hb % 2
            r = b * B + j
            ps_bc = psb.tile([COUT, 512], F32, tag="ps_bc")
            nc.tensor.matmul(ps_bc[:, 0:256],
                             e16_b[:, r * COUT:(r + 1) * COUT],
                             mru16[:], start=True, stop=True)
            nc.tensor.matmul(ps_bc[:, 256:512],
                             e16_b[:, (r + 1) * COUT:(r + 2) * COUT],
                             mru16[:], start=True, stop=True)
            mbc = mb.tile([COUT, 512], BF16, tag="mbc")
            nc.vector.tensor_copy(mbc[:], ps_bc[:])
            return mbc

        def conv_chunk(hb, j, ci, mbc, v16):
            K, b = hb // 2, hb % 2
            xm3 = xm_tiles[hb % 2][:, :].rearrange("p (r c) -> p r c", c=258)
            r = b * B + j
            ps_c = psc.tile([COUT, 512], F32, tag="ps_c")
            for ky in range(KS):
                nc.tensor.matmul(ps_c[:],
                                 wtp_b[:, ky * COUT:(ky + 1) * COUT],
                                 xm3[0:128, j + ky:j + ky + 2, 0:256],
                                 start=(ky == 0), stop=False)
            for ky in range(KS):
                nc.tensor.matmul(ps_c[:],
                                 wts_b[:, ky * COUT:(ky + 1) * COUT],
                                 xm3[0:64, j + ky:j + ky + 2, 2:258],
                                 start=False, stop=False)
            nc.tensor.matmul(ps_c[:, 0:256],
                             bps16_b[:, r * COUT:(r + 1) * COUT],
                             v16[:], start=False, stop=False)
            nc.tensor.matmul(ps_c[:, 256:512],
                             bps16_b[:, (r + 1) * COUT:(r + 2) * COUT],
                             v16[:], start=False, stop=True)
            row = b * HB + K * B + j
            if os.environ.get("DEBUG_MRU"):
                nc.vector.scalar_tensor_tensor(
                    out=prebn[:, row * W_:row * W_ + 512],
                    in0=ps_c[:], scalar=0.0, in1=mbc[:],
                    op0=ALU.mult, op1=ALU.add,
                    accum_out=sum_slots[:, ci:ci + 1])
            else:
                nc.vector.scalar_tensor_tensor(
                    out=prebn[:, row * W_:row * W_ + 512],
                    in0=ps_c[:], scalar=0.0, in1=mbc[:],
                    op0=ALU.add, op1=ALU.mult,
                    accum_out=sum_slots[:, ci:ci + 1])
            sq_scr = chk.tile([COUT, 512], BF16, tag="sq_scr")
            nc.scalar.activation(
                sq_scr[:], prebn[:, row * W_:row * W_ + 512], ACTF.Square,
                accum_out=sq_slots[:, ci:ci + 1])

        # ---- preamble: update path for block 0, xm for half-block 0 ----
        upd_part1(0)
        upd_part1(1)
        ps_u0 = upd_t3()
        mru_cur, v_cur = upd_part2(0, ps_u0)
        make_xm(0, 0)
        make_xm(0, 1)
        mbc_nxt = bcast_chunk(0, 0, mru_cur)

        ci = 0
        for hb in range(nhb):
            K, b = hb // 2, hb % 2
            if hb + 1 < nhb:
                x_tiles[hb + 1] = load_x(hb + 1)
            if hb + 3 < nhb:
                m_tiles[hb + 3] = load_m(hb + 3)
            if hb + 2 < nhb:
                upd_part1(hb + 2)
            mbc2 = bcast_chunk(hb, 2, mru_cur)
            conv_chunk(hb, 0, ci, mbc_nxt, v_cur); ci += 1
            if b == 1 and K + 1 < nblk:
                ps_u = upd_t3()
            mbc4 = bcast_chunk(hb, 4, mru_cur)
            conv_chunk(hb, 2, ci, mbc2, v_cur); ci += 1
            if b == 1 and K + 1 < nblk:
                mru_n, v_n = upd_part2(K + 1, ps_u)
            if hb + 1 < nhb:
                make_xm(hb + 1, 0)
            mbc6 = bcast_chunk(hb, 6, mru_cur)
            conv_chunk(hb, 4, ci, mbc4, v_cur); ci += 1
            if hb + 1 < nhb:
                nxt_mru = mru_n if (b == 1 and K + 1 < nblk) else mru_cur
                mbc_nxt = bcast_chunk(hb + 1, 0, nxt_mru)
            conv_chunk(hb, 6, ci, mbc6, v_cur); ci += 1
            if b == 1:
                if K + 1 < nblk:
                    mru_cur, v_cur = mru_n, v_n
            # drop refs to freed ring slots
            x_tiles.pop(hb, None)
            m_tiles.pop(hb, None)

        assert ci == nchunk

        # ---- BN stats: reduce, all-reduce, affine coeffs ----
        cc_sb = const.tile([COUT, 2], F32)
        nc.vector.tensor_reduce(cc_sb[:, 0:1], sum_slots[:],
                                axis=mybir.AxisListType.X, op=ALU.add)
        nc.vector.tensor_reduce(cc_sb[:, 1:2], sq_slots[:],
                                axis=mybir.AxisListType.X, op=ALU.add)
        nc.sync.dma_start(cc_in[:], cc_sb[:])
        nc.gpsimd.collective_compute(
            "AllReduce", ALU.add,
            replica_groups=[list(range(n_cores))],
            ins=[cc_in.ap().opt()], outs=[cc_out.ap().opt()])
        st_sb = const.tile([COUT, 2], F32)
        nc.sync.dma_start(st_sb[:], cc_out[:])
        mean_t = const.tile([COUT, 1], F32)
        nc.vector.tensor_scalar_mul(mean_t[:], st_sb[:, 0:1], 1.0 / TOT)
        e2_t = const.tile([COUT, 1], F32)
        nc.vector.tensor_scalar_mul(e2_t[:], st_sb[:, 1:2], 1.0 / TOT)
        msq_t = const.tile([COUT, 1], F32)
        nc.vector.tensor_mul(msq_t[:], mean_t[:], mean_t[:])
        var_t = const.tile([COUT, 1], F32)
        nc.vector.tensor_sub(var_t[:], e2_t[:], msq_t[:])
        std_t = const.tile([COUT, 1], F32)
        nc.scalar.activation(std_t[:], var_t[:], ACTF.Sqrt, bias=eps_t[:])
        rstd_t = const.tile([COUT, 1], F32)
        nc.vector.reciprocal(rstd_t[:], std_t[:])
        scale_t = const.tile([COUT, 1], F32)
        nc.vector.tensor_mul(scale_t[:], gam_t[:], rstd_t[:])
        tmp_t = const.tile([COUT, 1], F32)
        nc.vector.tensor_mul(tmp_t[:], mean_t[:], scale_t[:])
        bias_t = const.tile([COUT, 1], F32)
        nc.vector.tensor_sub(bias_t[:], bet_t[:], tmp_t[:])

        # ---- pass 2: out = relu(scale*prebn + bias), from SBUF bf16 ----
        # staging reuses the idle x/m ring buffers (4-deep pipeline)
        P2 = 2048
        for i in range(0, H * W_, P2):
            if (i // P2) % 4 == 0:
                o_full = iox.tile([128, nrows * 258], F32, tag="x_t")
                o_t = o_full[:, 0:P2]
            else:
                o_full = iom.tile([128, nrows * 258], F32R, tag="m_t")
                o_t = o_full[:, 0:P2].bitcast(F32)
            nc.scalar.activation(o_t, prebn[:, i:i + P2], ACTF.Relu,
                                 bias=bias_t[:], scale=scale_t[:])
            nc.sync.dma_start(OUT[:, i:i + P2], o_t)
            if DBG is not None:
                d_full = iox.tile([128, nrows * 258], F32, tag="x_t")
                d_t = d_full[:, 0:P2]
                nc.vector.tensor_copy(d_t, prebn[:, i:i + P2])
                nc.sync.dma_start(DBG[:, i:i + P2], d_t)

    return nc


def make_host_inputs(x_i, mask_i, W, b, gamma, beta, B=8):
    """Per-core in_map for one image shard (host-side layout prep)."""
    # wtp: pairs (ky,0) top / (ky,1) bottom; wts: singles (ky,2)
    Wt = np.asarray(W, np.float32)           # [COUT, CIN, 3, 3]
    wtp = np.zeros((128, KS * COUT), np.float32)
    wts_pack = np.zeros((128, 768), np.float32)
    for ky in range(KS):
        wtp[0:64, ky * COUT:(ky + 1) * COUT] = Wt[:, :, ky, 0].T
        wtp[64:128, ky * COUT:(ky + 1) * COUT] = Wt[:, :, ky, 1].T
        wts_pack[0:64, KS * COUT + ky * COUT:KS * COUT + (ky + 1) * COUT] = \
            Wt[:, :, ky, 2].T
    wts_pack[:, 0:KS * COUT] = wtp
    EB1p = np.zeros((128, 2048), np.float32)
    EB2p = np.zeros((128, 2048), np.float32)
    bf = np.asarray(b, np.float32) / SLIDE
    for r in range(16):
        EB1p[r, r * COUT:(r + 1) * COUT] = 1.0
        EB2p[r, r * COUT:(r + 1) * COUT] = bf
    half = np.zeros((128, 2), np.float32)
    half[0:64, :] = 1.0
    T3 = np.zeros((2 * (B + 2), 2 * B), np.float32)
    for band in range(2):
        for jj in range(B):
            for d in range(3):
                T3[band * (B + 2) + jj + d, band * B + jj] = 1.0
    return {
        "x": np.ascontiguousarray(x_i, dtype=np.float32),
        "mask": np.ascontiguousarray(mask_i, dtype=np.float32),
        "wts": wts_pack,
        "eb1": EB1p,
        "eb2": EB2p,
        "half": half,
        "t3": T3,
        "gam": gamma.reshape(COUT, 1).astype(np.float32),
        "bet": beta.reshape(COUT, 1).astype(np.float32),
    }


_NC_CACHE = {}


def kernel(x, mask, W, b, gamma, beta):
    x = np.asarray(x)
    mask = np.asarray(mask)
    W = np.asarray(W)
    b = np.asarray(b)
    gamma = np.asarray(gamma)
    beta = np.asarray(beta)
    N, _, H, _ = x.shape
    n_cores = N
    key = (n_cores, H, bool(os.environ.get("DEBUG_PREBN")), bool(os.environ.get("DEBUG_MRU")))
    if key not in _NC_CACHE:
        nc = build_nc(n_cores=n_cores, H=H)
        nc.finalize()
        _NC_CACHE[key] = nc
    nc = _NC_CACHE[key]

    in_maps = [make_host_inputs(x[i], mask[i], W, b, gamma, beta)
               for i in range(n_cores)]
    res = run_bass_kernel_spmd(nc, in_maps, core_ids=list(range(n_cores)),
                               trace=bool(os.environ.get("KERNEL_TRACE")))
    out = np.stack([res.results[i]["out"].reshape(COUT, H, W_)
                    for i in range(n_cores)])
    if os.environ.get("DEBUG_PREBN"):
        kernel.dbg = np.stack([res.results[i]["dbg"].reshape(COUT, H, W_)
                               for i in range(n_cores)])
    upd = np.stack([res.results[i]["upd"] for i in range(n_cores)])
    update_full = np.broadcast_to(upd[:, None, :, :], (N, COUT, H, W_))
    kernel.last_result = res
    return out, update_full


# revision 32
# speedup vs baseline: 1.0653x; 1.0653x over previous
"""Trainium2 Bass kernel for nn_PartialConvLayer (partial conv 3x3 + mask
update + BatchNorm(batch stats) + ReLU), data-parallel over batch on 8 cores.

Math (per image):
  update = conv(mask, ones(Cin,3,3)), pad 1          # integer in {0..576}
  u      = clip(update, 0, 1)                        # exactly binary
  mr     = 576 / (update + 1e-6) * u
  conv   = conv(x*mask, W), pad 1                    # no bias
  prebn  = conv * mr * u + b * u
  BN over (N,H,W) batch stats (all-reduced across cores), then ReLU.
Returns (out, broadcast(update_clipped)).

v3 design (on top of v2's balanced DMA + SBUF-resident bf16 prebn):
  - 128-partition contraction matmuls run ~1.7x faster than 64 on TRN2's PE,
    so the conv uses a dual xm layout: partitions 0:64 hold x*m with a left
    guard column (tap kx reads window [kx, kx+256)), partitions 64:128 hold
    the same band's x*m unshifted (same window gives tap kx+1). Taps pair as
    (ky,0)+(ky,1) in one 128-contraction matmul; (ky,2) stays 64-contraction.
    The duplicate halves come from loading x and mask twice per half-block.
  - Each half-block = one band x 8 rows = 4 chunks of [COUT, 512].
  - Per-chunk mask scalars via zero-padded 128-row selector matmuls:
    e8[p, q*128+o] = d(p,q) broadcasts mru pair-strips (mru8[128,512],
    rows 0:8 = strips, rest zeros) into PSUM in ONE matmul; bps8 folds
    b[o]/576 * v into the conv PSUM in ONE matmul.
  - Update path (ones-matmul channel sums -> sb->sb scatter -> T3 vertical
    sum -> DVE math) runs one block ahead of the conv consuming it.
"""
import os
import numpy as np
from contextlib import ExitStack

import concourse.bass as bass
import concourse.tile as tile
from concourse import mybir, bacc
from concourse import library_config
from concourse.bass_utils import run_bass_kernel_spmd

F32 = mybir.dt.float32
F32R = mybir.dt.float32r
BF16 = mybir.dt.bfloat16
ALU = mybir.AluOpType
ACTF = mybir.ActivationFunctionType

CIN = 64
COUT = 128
W_ = 256
KS = 3
EPS_MASK = 1e-6
EPS_BN = 1e-5
SLIDE = float(CIN * KS * KS)   # 576


def build_nc(n_cores=8, H=256, B=8):
    """SPMD program for one core holding one [CIN, H, W_] image."""
    HB = H // 2                      # rows per band
    nblk = HB // B                   # 16 blocks
    nhb = 2 * nblk                   # 32 half-blocks (block K, band b)
    nrows = B + 2                    # rows per band tile (with halo)
    nchunk = (H * W_) // 512         # 128 chunks per core
    TOT = float(n_cores * H * W_)    # BN count

    nc = bacc.Bacc(None, num_devices=n_cores)

    X = nc.dram_tensor("x", [CIN, H, W_], F32, kind="ExternalInput")
    M = nc.dram_tensor("mask", [CIN, H, W_], F32R, kind="ExternalInput")
    # wtp [128, 384] (tap pairs) | wts [64, 384] (kx=2 singles) packed
    WTS = nc.dram_tensor("wts", [128, 768], F32R, kind="ExternalInput")
    EB1 = nc.dram_tensor("eb1", [128, 2048], F32R, kind="ExternalInput")
    EB2 = nc.dram_tensor("eb2", [128, 2048], F32R, kind="ExternalInput")
    HALF = nc.dram_tensor("half", [128, 2], F32R, kind="ExternalInput")
    T3 = nc.dram_tensor("t3", [2 * nrows, 2 * B], F32R, kind="ExternalInput")
    GAM = nc.dram_tensor("gam", [COUT, 1], F32, kind="ExternalInput")
    BET = nc.dram_tensor("bet", [COUT, 1], F32, kind="ExternalInput")

    OUT = nc.dram_tensor("out", [COUT, H * W_], F32, kind="ExternalOutput")
    DBG = (nc.dram_tensor("dbg", [COUT, H * W_], F32, kind="ExternalOutput")
           if os.environ.get("DEBUG_PREBN") else None)

    UPD = nc.dram_tensor("upd", [H, W_], F32, kind="ExternalOutput")

    cc_in = nc.dram_tensor("ccin", [COUT, 2], F32)
    cc_out = nc.dram_tensor("ccout", [COUT, 2], F32,
                            addr_space="Shared" if n_cores > 4 else "Local")

    with tile.TileContext(nc) as tc, ExitStack() as ctx:
        nc.gpsimd.load_library(library_config.mlp)

        const = ctx.enter_context(tc.tile_pool(name="const", bufs=1))
        iox = ctx.enter_context(tc.tile_pool(name="iox", bufs=1))
        iom = ctx.enter_context(tc.tile_pool(name="iom", bufs=3))
        up1 = ctx.enter_context(tc.tile_pool(name="up1", bufs=1))
        up2 = ctx.enter_context(tc.tile_pool(name="up2", bufs=2))
        chk = ctx.enter_context(tc.tile_pool(name="chk", bufs=1))
        mb = ctx.enter_context(tc.tile_pool(name="mb", bufs=2))
        psc = ctx.enter_context(tc.tile_pool(name="psc", bufs=4, space="PSUM"))
        psb = ctx.enter_context(tc.tile_pool(name="psb", bufs=2, space="PSUM"))
        pss = ctx.enter_context(tc.tile_pool(name="pss", bufs=1, space="PSUM"))
        psu = ctx.enter_context(tc.tile_pool(name="psu", bufs=1, space="PSUM"))

        # ---- first x load starts immediately; consts stage via m slots ----
        # x/m staged pre-padded and pre-shifted: [0:64] data at col 1 (guards
        # 0,257), [64:128] data at col 0 (guards 256,257), 258-pitch rows.
        def load_dup(tens, tl, k, b, first):
            r0 = k * B
            base = r0 + b * HB
            lo = max(base - 1, 0)
            hi = min(base + B + 1, H)
            n = hi - lo
            sl0 = lo - (base - 1)
            if first:
                nc.vector.memset(tl[:, :].bitcast(F32), 0.0)
            tl3 = tl[:, :].rearrange("p (r c) -> p r c", c=258)
            for h in range(2):
                nc.sync.dma_start(
                    tl3[64 * h:64 * h + 64, sl0:sl0 + n, 1 - h:257 - h],
                    bass.AP(tensor=tens, offset=lo * W_,
                            ap=[[H * W_, CIN], [W_, n], [1, W_]]))
            if not first:
                if sl0 > 0:
                    nc.vector.memset(tl[:, 0:sl0 * 258].bitcast(F32), 0.0)
                if sl0 + n < nrows:
                    nc.vector.memset(
                        tl[:, (sl0 + n) * 258:nrows * 258].bitcast(F32), 0.0)

        def load_x(hb):
            x_t = iox.tile([128, nrows * 258], F32, tag="x_t")
            load_dup(X, x_t, hb // 2, hb % 2, hb < 1)
            return x_t

        def load_m(hb):
            m_t = iom.tile([128, nrows * 258], F32R, tag="m_t")
            load_dup(M, m_t, hb // 2, hb % 2, hb < 3)
            return m_t

        m_first = [None]

        # m(0) + x(0) loads start immediately; consts stage via later slots
        m_first[0] = load_m(0)
        x_tiles = {0: load_x(0)}
        stg = iom.tile([128, nrows * 258], F32R, tag="m_t")
        nc.sync.dma_start(stg[:, 0:768], WTS[:])
        wtp_b = const.tile([128, KS * COUT], BF16)
        nc.vector.tensor_copy(wtp_b[:], stg[:, 0:KS * COUT])
        wts_b = const.tile([64, KS * COUT], BF16)
        nc.vector.tensor_copy(wts_b[:], stg[0:64, KS * COUT:2 * KS * COUT])
        stg2 = iom.tile([128, nrows * 258], F32R, tag="m_t")
        nc.sync.dma_start(stg2[:, 0:2048], EB1[:])
        e16_b = const.tile([128, 16 * COUT], BF16)
        nc.vector.tensor_copy(e16_b[:], stg2[:, 0:16 * COUT])
        stg3 = iom.tile([128, nrows * 258], F32R, tag="m_t")
        nc.sync.dma_start(stg3[:, 0:2048], EB2[:])
        bps16_b = const.tile([128, 16 * COUT], BF16)
        nc.vector.tensor_copy(bps16_b[:], stg3[:, 0:16 * COUT])
        t3_r = const.tile([2 * nrows, 2 * B], F32R)
        nc.sync.dma_start(t3_r[:], T3[:])

        half_t = const.tile([128, 2], F32R)
        nc.sync.dma_start(half_t[:], HALF[:])
        gam_t = const.tile([COUT, 1], F32)
        nc.sync.dma_start(gam_t[:], GAM[:])
        bet_t = const.tile([COUT, 1], F32)
        nc.sync.dma_start(bet_t[:], BET[:])
        eps_t = const.tile([COUT, 1], F32)
        nc.vector.memset(eps_t[:], EPS_BN)
        sum_slots = const.tile([COUT, nchunk], F32)
        sq_slots = const.tile([COUT, nchunk], F32)

        # dual xm ring: top half guard cols {0,257}, bottom {256,257}
        xm_tiles = []
        for i in range(2):
            t = const.tile([128, nrows * 258], BF16, tag=f"xm{i}")
            nc.vector.memset(t[:].bitcast(F32), 0.0)
            xm_tiles.append(t)
        s_rows = const.tile([2 * nrows, 258], F32R)
        nc.vector.memset(s_rows[:].bitcast(F32), 0.0)

        prebn = const.tile([128, H * W_], BF16)

        m_tiles = {0: m_first[0], 1: load_m(1), 2: load_m(2)}

        def upd_part1(hb):
            """Channel sums of mask band hb -> scatter into s_rows rows."""
            b = hb % 2
            m3 = m_tiles[hb][:, :].rearrange("p (r c) -> p r c", c=258)
            for p in range(nrows // 2):
                ps_s = pss.tile([2, 512], F32, tag="ps_s")
                nc.tensor.matmul(ps_s[:], half_t[:],
                                 m3[:, 2 * p:2 * p + 2, 1:257],
                                 start=True, stop=True)
                s_pair = up1.tile([1, 512], F32R, tag="s_pair")
                nc.scalar.copy(s_pair[:], ps_s[0:1, :])
                nc.sync.dma_start(
                    s_rows[b * nrows + 2 * p:b * nrows + 2 * p + 2, 1:257],
                    s_pair[0:1, :].rearrange("p (r c) -> p r c", c=256))

        def upd_t3():
            ps_u = psu.tile([2 * B, 258], F32, tag="ps_u")
            nc.tensor.matmul(ps_u[:], t3_r[:], s_rows[:], start=True, stop=True)
            return ps_u

        def upd_part2(K, ps_u):
            """Horizontal sum + update math on [16, 256] for block K."""
            r0 = K * B
            u_sb = up1.tile([2 * B, 258], F32, tag="u_sb")
            nc.scalar.copy(u_sb[:], ps_u[:])
            vh = up1.tile([2 * B, W_], F32, tag="vh")
            nc.vector.tensor_add(vh[:], u_sb[:, 0:256], u_sb[:, 1:257])
            nc.vector.tensor_add(vh[:], vh[:], u_sb[:, 2:258])
            u_clip = up1.tile([2 * B, W_], F32, tag="u_clip")
            nc.vector.tensor_scalar_min(u_clip[:], vh[:], 1.0)
            upde = up1.tile([2 * B, W_], F32, tag="upde")
            nc.vector.tensor_scalar_add(upde[:], vh[:], EPS_MASK)
            rec = up1.tile([2 * B, W_], F32, tag="vh")
            nc.vector.reciprocal(rec[:], upde[:])
            mru16f = up1.tile([2 * B, W_], F32, tag="mru16f")
            nc.vector.scalar_tensor_tensor(
                out=mru16f[:], in0=rec[:], scalar=SLIDE, in1=u_clip[:],
                op0=ALU.mult, op1=ALU.mult)
            v16f = up1.tile([2 * B, W_], F32, tag="v16f")
            nc.vector.scalar_tensor_tensor(
                out=v16f[:], in0=upde[:], scalar=1.0, in1=u_clip[:],
                op0=ALU.mult, op1=ALU.mult)
            mru16 = up2.tile([128, W_], BF16, tag="mru16")
            if K < 2:
                nc.vector.memset(mru16[:].bitcast(F32), 0.0)
            nc.vector.tensor_copy(mru16[0:2 * B, :], mru16f[:])
            v16 = up2.tile([128, W_], BF16, tag="v16")
            if K < 2:
                nc.vector.memset(v16[:].bitcast(F32), 0.0)
            nc.vector.tensor_copy(v16[0:2 * B, :], v16f[:])
            nc.sync.dma_start(
                bass.AP(tensor=UPD, offset=r0 * W_,
                        ap=[[HB * W_, 2], [W_, B], [1, W_]]),
                u_clip[:])
            return mru16, v16

        def make_xm(hb, half):
            """One contiguous full-width multiply (inputs pre-shifted)."""
            if half != 0:
                return xm_tiles[hb % 2]
            xm = xm_tiles[hb % 2]
            nc.vector.tensor_tensor(
                xm[:, :], x_tiles[hb][:, :], m_tiles[hb][:, :], op=ALU.mult)
            return xm

        def bcast_chunk(hb, j, mru16):
            b = hb % 2
            r = b * B + j
            ps_bc = psb.tile([COUT, 512], F32, tag="ps_bc")
            nc.tensor.matmul(ps_bc[:, 0:256],
                             e16_b[:, r * COUT:(r + 1) * COUT],
                             mru16[:], start=True, stop=True)
            nc.tensor.matmul(ps_bc[:, 256:512],
                             e16_b[:, (r + 1) * COUT:(r + 2) * COUT],
                             mru16[:], start=True, stop=True)
            mbc = mb.tile([COUT, 512], BF16, tag="mbc")
            nc.vector.tensor_copy(mbc[:], ps_bc[:])
            return mbc

        def conv_chunk(hb, j, ci, mbc, v16):
            K, b = hb // 2, hb % 2
            xm3 = xm_tiles[hb % 2][:, :].rearrange("p (r c) -> p r c", c=258)
            r = b * B + j
            ps_c = psc.tile([COUT, 512], F32, tag="ps_c")
            for ky in range(KS):
                nc.tensor.matmul(ps_c[:],
                                 wtp_b[:, ky * COUT:(ky + 1) * COUT],
                                 xm3[0:128, j + ky:j + ky + 2, 0:256],
                                 start=(ky == 0), stop=False)
            for ky in range(KS):
                nc.tensor.matmul(ps_c[:],
                                 wts_b[:, ky * COUT:(ky + 1) * COUT],
                                 xm3[0:64, j + ky:j + ky + 2, 2:258],
                                 start=False, stop=False)
            nc.tensor.matmul(ps_c[:, 0:256],
                             bps16_b[:, r * COUT:(r + 1) * COUT],
                             v16[:], start=False, stop=False)
            nc.tensor.matmul(ps_c[:, 256:512],
                             bps16_b[:, (r + 1) * COUT:(r + 2) * COUT],
                             v16[:], start=False, stop=True)
            row = b * HB + K * B + j
            if os.environ.get("DEBUG_MRU"):
                nc.vector.scalar_tensor_tensor(
                    out=prebn[:, row * W_:row * W_ + 512],
                    in0=ps_c[:], scalar=0.0, in1=mbc[:],
                    op0=ALU.mult, op1=ALU.add,
                    accum_out=sum_slots[:, ci:ci + 1])
            else:
                nc.vector.scalar_tensor_tensor(
                    out=prebn[:, row * W_:row * W_ + 512],
                    in0=ps_c[:], scalar=0.0, in1=mbc[:],
                    op0=ALU.add, op1=ALU.mult,
                    accum_out=sum_slots[:, ci:ci + 1])
            sq_scr = chk.tile([COUT, 512], BF16, tag="sq_scr")
            nc.scalar.activation(
                sq_scr[:], prebn[:, row * W_:row * W_ + 512], ACTF.Square,
                accum_out=sq_slots[:, ci:ci + 1])

        # ---- preamble: update path for block 0, xm for half-block 0 ----
        upd_part1(0)
        upd_part1(1)
        ps_u0 = upd_t3()
        mru_cur, v_cur = upd_part2(0, ps_u0)
        make_xm(0, 0)
        make_xm(0, 1)
        mbc_nxt = bcast_chunk(0, 0, mru_cur)

        ci = 0
        for hb in range(nhb):
            K, b = hb // 2, hb % 2
            if hb + 1 < nhb:
                x_tiles[hb + 1] = load_x(hb + 1)
            if hb + 3 < nhb:
                m_tiles[hb + 3] = load_m(hb + 3)
            if hb + 2 < nhb:
                upd_part1(hb + 2)
            mbc2 = bcast_chunk(hb, 2, mru_cur)
            conv_chunk(hb, 0, ci, mbc_nxt, v_cur); ci += 1
            if b == 1 and K + 1 < nblk:
                ps_u = upd_t3()
            mbc4 = bcast_chunk(hb, 4, mru_cur)
            conv_chunk(hb, 2, ci, mbc2, v_cur); ci += 1
            if b == 1 and K + 1 < nblk:
                mru_n, v_n = upd_part2(K + 1, ps_u)
            if hb + 1 < nhb:
                make_xm(hb + 1, 0)
            mbc6 = bcast_chunk(hb, 6, mru_cur)
            conv_chunk(hb, 4, ci, mbc4, v_cur); ci += 1
            if hb + 1 < nhb:
                nxt_mru = mru_n if (b == 1 and K + 1 < nblk) else mru_cur
                mbc_nxt = bcast_chunk(hb + 1, 0, nxt_mru)
            conv_chunk(hb, 6, ci, mbc6, v_cur); ci += 1
            if b == 1:
                if K + 1 < nblk:
                    mru_cur, v_cur = mru_n, v_n
            # drop refs to freed ring slots
            x_tiles.pop(hb, None)
            m_tiles.pop(hb, None)

        assert ci == nchunk

        # ---- BN stats: reduce, all-reduce, affine coeffs ----
        cc_sb = const.tile([COUT, 2], F32)
        nc.vector.tensor_reduce(cc_sb[:, 0:1], sum_slots[:],
                                axis=mybir.AxisListType.X, op=ALU.add)
        nc.vector.tensor_reduce(cc_sb[:, 1:2], sq_slots[:],
                                axis=mybir.AxisListType.X, op=ALU.add)
        nc.sync.dma_start(cc_in[:], cc_sb[:])
        nc.gpsimd.collective_compute(
            "AllReduce", ALU.add,
            replica_groups=[list(range(n_cores))],
            ins=[cc_in.ap().opt()], outs=[cc_out.ap().opt()])
        st_sb = const.tile([COUT, 2], F32)
        nc.sync.dma_start(st_sb[:], cc_out[:])
        mean_t = const.tile([COUT, 1], F32)
        nc.vector.tensor_scalar_mul(mean_t[:], st_sb[:, 0:1], 1.0 / TOT)
        e2_t = const.tile([COUT, 1], F32)
        nc.vector.tensor_scalar_mul(e2_t[:], st_sb[:, 1:2], 1.0 / TOT)
        msq_t = const.tile([COUT, 1], F32)
        nc.vector.tensor_mul(msq_t[:], mean_t[:], mean_t[:])
        var_t = const.tile([COUT, 1], F32)
        nc.vector.tensor_sub(var_t[:], e2_t[:], msq_t[:])
        std_t = const.tile([COUT, 1], F32)
        nc.scalar.activation(std_t[:], var_t[:], ACTF.Sqrt, bias=eps_t[:])
        rstd_t = const.tile([COUT, 1], F32)
        nc.vector.reciprocal(rstd_t[:], std_t[:])
        scale_t = const.tile([COUT, 1], F32)
        nc.vector.tensor_mul(scale_t[:], gam_t[:], rstd_t[:])
        tmp_t = const.tile([COUT, 1], F32)
        nc.vector.tensor_mul(tmp_t[:], mean_t[:], scale_t[:])
        bias_t = const.tile([COUT, 1], F32)
        nc.vector.tensor_sub(bias_t[:], bet_t[:], tmp_t[:])

        # ---- pass 2: out = relu(scale*prebn + bias), from SBUF bf16 ----
        # staging reuses the idle x/m ring buffers (4-deep pipeline)
        P2 = 2048
        for i in range(0, H * W_, P2):
            if (i // P2) % 4 == 0:
                o_full = iox.tile([128, nrows * 258], F32, tag="x_t")
                o_t = o_full[:, 0:P2]
            else:
                o_full = iom.tile([128, nrows * 258], F32R, tag="m_t")
                o_t = o_full[:, 0:P2].bitcast(F32)
            nc.scalar.activation(o_t, prebn[:, i:i + P2], ACTF.Relu,
                                 bias=bias_t[:], scale=scale_t[:])
            nc.sync.dma_start(OUT[:, i:i + P2], o_t)
            if DBG is not None:
                d_full = iox.tile([128, nrows * 258], F32, tag="x_t")
                d_t = d_full[:, 0:P2]
                nc.vector.tensor_copy(d_t, prebn[:, i:i + P2])
                nc.sync.dma_start(DBG[:, i:i + P2], d_t)

    return nc


def make_host_inputs(x_i, mask_i, W, b, gamma, beta, B=8):
    """Per-core in_map for one image shard (host-side layout prep)."""
    # wtp: pairs (ky,0) top / (ky,1) bottom; wts: singles (ky,2)
    Wt = np.asarray(W, np.float32)           # [COUT, CIN, 3, 3]
    wtp = np.zeros((128, KS * COUT), np.float32)
    wts_pack = np.zeros((128, 768), np.float32)
    for ky in range(KS):
        wtp[0:64, ky * COUT:(ky + 1) * COUT] = Wt[:, :, ky, 0].T
        wtp[64:128, ky * COUT:(ky + 1) * COUT] = Wt[:, :, ky, 1].T
        wts_pack[0:64, KS * COUT + ky * COUT:KS * COUT + (ky + 1) * COUT] = \
            Wt[:, :, ky, 2].T
    wts_pack[:, 0:KS * COUT] = wtp
    EB1p = np.zeros((128, 2048), np.float32)
    EB2p = np.zeros((128, 2048), np.float32)
    bf = np.asarray(b, np.float32) / SLIDE
    for r in range(16):
        EB1p[r, r * COUT:(r + 1) * COUT] = 1.0
        EB2p[r, r * COUT:(r + 1) * COUT] = bf
    half = np.zeros((128, 2), np.float32)
    half[0:64, :] = 1.0
    T3 = np.zeros((2 * (B + 2), 2 * B), np.float32)
    for band in range(2):
        for jj in range(B):
            for d in range(3):
                T3[band * (B + 2) + jj + d, band * B + jj] = 1.0
    return {
        "x": np.ascontiguousarray(x_i, dtype=np.float32),
        "mask": np.ascontiguousarray(mask_i, dtype=np.float32),
        "wts": wts_pack,
        "eb1": EB1p,
        "eb2": EB2p,
        "half": half,
        "t3": T3,
        "gam": gamma.reshape(COUT, 1).astype(np.float32),
        "bet": beta.reshape(COUT, 1).astype(np.float32),
    }


_NC_CACHE = {}


def kernel(x, mask, W, b, gamma, beta):
    x = np.asarray(x)
    mask = np.asarray(mask)
    W = np.asarray(W)
    b = np.asarray(b)
    gamma = np.asarray(gamma)
    beta = np.asarray(beta)
    N, _, H, _ = x.shape
    n_cores = N
    key = (n_cores, H, bool(os.environ.get("DEBUG_PREBN")), bool(os.environ.get("DEBUG_MRU")))
    if key not in _NC_CACHE:
        nc = build_nc(n_cores=n_cores, H=H)
        nc.finalize()
        _NC_CACHE[key] = nc
    nc = _NC_CACHE[key]

    in_maps = [make_host_inputs(x[i], mask[i], W, b, gamma, beta)
               for i in range(n_cores)]
    res = run_bass_kernel_spmd(nc, in_maps, core_ids=list(range(n_cores)),
                               trace=bool(os.environ.get("KERNEL_TRACE")))
    out = np.stack([res.results[i]["out"].reshape(COUT, H, W_)
                    for i in range(n_cores)])
    if os.environ.get("DEBUG_PREBN"):
        kernel.dbg = np.stack([res.results[i]["dbg"].reshape(COUT, H, W_)
                               for i in range(n_cores)])
    upd = np.stack([res.results[i]["upd"] for i in range(n_cores)])
    update_full = np.broadcast_to(upd[:, None, :, :], (N, COUT, H, W_))
    kernel.last_result = res
    return out, update_full
